# revision 19
# baseline (speedup 1.0000x reference)
"""Ewald reciprocal-space sum on 8 Trainium2 NeuronCores.

Math: for each system b, S(k) = sum_n q_n e^{i k.r_n} over the integer
k-grid n in [-10,10]^3, k = n @ G, G = 2*pi*inv(cell)^T. The weight mask
keeps only k_sq <= (2*pi/DL)^2, i.e. |n|^2 <= 100, and one hemisphere.

Factorization used here: k.r = 2*pi*(n1*phi1 + n2*phi2 + n3*phi3) with
phi_d = (r @ inv(cell))_d, so
  S[n1,n2,n3] = sum_a (q_a e^{i 2pi n3 phi3}) * e^{i 2pi(n1 phi1 + n2 phi2)}.
The (n1,n2) pair table (n1 in [0,10] half-grid, sphere-pruned to two
rectangular blocks, 207 pairs x {sin,cos} = 414 cols) is built on device:
one fused DVE add+wrap per block per 128-atom chunk, then one ACT Sin,
then one PE matmul per chunk against the 42-col stationary side
qv = [-q sin(2pi n3 phi3) | q cos(2pi n3 phi3)], accumulating
PSUM[42, 414] over the 8 chunks.

Host does O(N) prep (centered fractional phases frac(j*phi), the qv
table) and the tiny O(K) weighted reduction mirroring the reference,
summing partial S across the core pair of each system before squaring.
Each core owns half the atoms of system c//2.
"""

import numpy as np

# ---- problem constants (hardcoded per contract) ----
B = 4
N_PER = 2000
NK = 10
DL = 2.0
SIGMA = 1.0
EPS = 1e-6
NORM = 90.0474
TWOPI = 2.0 * np.pi

N_CORES = 8
CORES_PER_SYS = 2
ATOMS_PER_CORE = (B * N_PER) // N_CORES     # 1000
CHUNKS = 8                                  # ceil(1000/128)
PADN = CHUNKS * 128                         # 1024

# (n1_lo, n1_hi, n2_lo, n2_hi) inclusive. A single full rect: pruning the
# |n|^2>100 corners costs a second DVE op per chunk whose fixed overhead
# exceeds the column savings on the bottleneck engine.
BLOCKS = [(0, 10, -10, 10)]
NP_PAIRS = sum((a1 - a0 + 1) * (b1 - b0 + 1) for a0, a1, b0, b1 in BLOCKS)
NCOL = 2 * NP_PAIRS                         # 414: (n1,h={sin,cos}) x n2
CPC = 85                                    # input cols/chunk: f1x 22|f2c 21|qv 42
NIN = CHUNKS * CPC                          # 680
# |scale|*0.5 must stay <= pi in fp32 (ACT Sin domain); 6.283185 < 2*pi
SCALE = -6.283185

_CACHE = {}


def _build_nc():
    import concourse.bacc as bacc
    import concourse.mybir as mybir
    import concourse.tile as tile

    # cheaper TileContext exit: the Bass preamble re-clears the whole
    # kernel sem range at every execution, so the exit-time sem clear and
    # second all-engine barrier are redundant for this single-context
    # kernel; keep drain + one barrier.
    def _cheap_drain_and_barrier(self, tick_clock, wait_clock):
        drain_inst = self.nc.sync.drain()
        wait_clock.add_sem_waits(
            drain_inst.ins, tile.ScopedClock({None: tick_clock.global_clock})
        )
        popped = self.nc._tile_sem_poison_stack.pop()
        assert popped is self._sem_poison

    f16 = mybir.dt.float16
    f32 = mybir.dt.float32
    Act = mybir.ActivationFunctionType

    # fused custom DVE op: out = wrap(in0 + in1 + s0) into [-s1, s1] with
    # period 1 (turn space)
    import concourse.dve_ops as dve_ops

    if not hasattr(dve_ops, "ADD_WRAP_EWALD"):
        from concourse.dve_spec import C0, C1, Spec, Src0, Src1, lower
        from concourse.dve_uop import DveOpSpec

        _y = (Src0 + Src1) + C0

        def _ref(in0, in1, s0, s1, imm2):
            y = in0 + in1 + s0
            return y + (
                (y < -s1).astype(np.float32) - (y > s1).astype(np.float32)
            )

        _spec = Spec(body=_y + ((_y < -C1) - (_y > C1)), reference=_ref)
        _shas = {
            ver: DveOpSpec(
                name="ADD_WRAP_EWALD", opcode=0,
                uops=lower(_spec, ver=ver), rd1_en=True,
            ).sha(ver)
            for ver in ("v3", "v4")
        }
        _op = dve_ops.DveOp("ADD_WRAP_EWALD", _spec, subdim=False, uops_sha=_shas)
        dve_ops.OPS.append(_op)
        dve_ops._SUB_OPCODE_FOR_NAME[_op.name] = (
            dve_ops._CUSTOM_DVE_ROW_BASE + len(dve_ops.OPS) - 1
        )
        dve_ops.CUSTOM_DVE_SPECS[_op.name] = _spec
        dve_ops.ADD_WRAP_EWALD = _op
    AW = dve_ops.ADD_WRAP_EWALD

    tile.TileContext._drain_and_barrier = _cheap_drain_and_barrier

    # Skip the all-engine barrier at the end of Bass.__init__ (~0.8us of
    # body-entry delay): the only cross-engine dependency it protects is
    # const-AP memsets (gpsimd, done ~6us) vs their first reader (the
    # first SIN's bias read at ~9us) -- a 3us margin.
    import concourse.bass as bass_mod
    _orig_barrier = bass_mod.Bass.all_engine_barrier
    bass_mod.Bass.all_engine_barrier = lambda self, *a, **k: None
    try:
        nc = bacc.Bacc(None, target_bir_lowering=False)
    finally:
        bass_mod.Bass.all_engine_barrier = _orig_barrier

    inp = nc.dram_tensor("inp", [128, NIN], f16, kind="ExternalInput")
    sout = nc.dram_tensor("sout", [42, NCOL], f16, kind="ExternalOutput")
    # raw (non-tile) SBUF staging for the output, so the post-TileContext
    # DMA below gets a concrete access pattern
    so = nc.alloc_sbuf_tensor("so_stage", [42, NCOL], f16)
    warm = nc.alloc_sbuf_tensor("warm", [128, 512], f16)
    out_sem = nc.alloc_semaphore("out_dma_sem")

    # input DMA split: chunk 0 alone (smallest latency to first compute).
    # Never issue DMAs from scalar: an Act-engine HWDGE clobbers the
    # activation table and forces a 1.3us table reload before the sins.
    SPLITS = [(0, 1, "sync"), (1, 4, "gpsimd"), (4, 8, "gpsimd")]
    HCOL = (NCOL // 2 + 1) & ~1                 # output col split, even

    with tile.TileContext(nc) as tc:
        with (
            tc.tile_pool(name="const", bufs=1) as cp,
            tc.tile_pool(name="work", bufs=3) as wp,
            tc.tile_pool(name="psum", bufs=1, space="PSUM") as pp,
        ):
            tiles = {}
            for si, (c0, c1, eng) in enumerate(SPLITS):
                # distinct names: same-named tiles share one buffer ring
                # (tag = assignee name) and would alias
                IN = cp.tile([128, (c1 - c0) * CPC], f16, name=f"IN{si}")
                getattr(nc, eng).dma_start(
                    out=IN[:], in_=inp[:, c0 * CPC : c1 * CPC]
                )
                for t in range(c0, c1):
                    tiles[t] = (IN, (t - c0) * CPC)

            ps = pp.tile([42, NCOL], f32)

            # (ah_lo, ah_hi) sub-ranges of the 22 interleaved (n1,h) f1x
            # cols; first and last chunks run in two halves to shorten
            # pipeline fill and drain.
            FULL = [(0, 22)]
            HALVES = [(0, 12), (12, 22)]
            for t in range(CHUNKS):
                IN, base = tiles[t]
                usrc = wp.tile([128, NCOL], f16)
                pieces = HALVES if t in (0, CHUNKS - 1) else FULL
                for pi, (ja, jb) in enumerate(pieces):
                    off = 21 * ja
                    nc.vector._custom_dve(
                        AW,
                        out=usrc[:, off : off + 21 * (jb - ja)].rearrange(
                            "p (ah b) -> p ah b", b=21
                        ),
                        in0=IN[:, base + ja : base + jb]
                        .unsqueeze(2)
                        .broadcast_to([128, jb - ja, 21]),
                        in1=IN[:, base + 22 : base + 43]
                        .unsqueeze(1)
                        .broadcast_to([128, jb - ja, 21]),
                        s0=0.0, s1=0.5,
                    )
                AA = wp.tile([128, NCOL], f16)
                for pi, (ja, jb) in enumerate(pieces):
                    off = 21 * ja
                    w = 21 * (jb - ja)
                    nc.scalar.activation(
                        out=AA[:, off : off + w], in_=usrc[:, off : off + w],
                        func=Act.Sin, bias=0.0, scale=SCALE,
                    )
                # chunk 0 must be a single full matmul: start=True resets
                # the whole PSUM bank, so col-split start pieces would wipe
                # each other. Chunk 7 pieces accumulate -> safe to split.
                mm_pieces = pieces if t == CHUNKS - 1 else FULL
                for ja, jb in mm_pieces:
                    off = 21 * ja
                    w = 21 * (jb - ja)
                    nc.tensor.matmul(
                        out=ps[:, off : off + w],
                        lhsT=IN[:, base + 43 : base + CPC],
                        rhs=AA[:, off : off + w],
                        start=(t == 0), stop=(t == CHUNKS - 1),
                        skip_group_check=True,
                    )

            # single PSUM->SBUF fp16 copy (a split copy across two engines
            # gets serialized by the raw-tensor write tracking)
            nc.scalar.activation(out=so.ap(), in_=ps[:], func=Act.Copy)

    # Output DMA outside the TileContext: the exit drain then only waits
    # for the copies (engine sems), not the DMA-completion semaphore
    # (~0.9us propagation); sync's program order puts the DMA after its
    # exit drain, and the NEFF epilogue drains the queue itself.
    nc.sync.dma_start(out=sout[:], in_=so.ap()).then_inc(out_sem, 16)

    nc.compile()
    return nc


def _get_nc():
    if "nc" not in _CACHE:
        _CACHE["nc"] = _build_nc()
    return _CACHE["nc"]


def _cf(x):
    """centered frac: ((x+0.5) mod 1) - 0.5 in [-0.5, 0.5)"""
    return ((x + 0.5) % 1.0) - 0.5


def _host_inputs(q, r, cell):
    """Per-core phase/qv tables in SBUF layout, fp16."""
    j1 = np.arange(11)
    n2r = np.arange(-10, 11)
    n3r = np.arange(-10, 11)
    in_maps = []
    for c in range(N_CORES):
        b = c // CORES_PER_SYS
        half = c % CORES_PER_SYS
        lo = b * N_PER + half * ATOMS_PER_CORE
        rs = r[lo : lo + ATOMS_PER_CORE].astype(np.float64)
        qs = q[lo : lo + ATOMS_PER_CORE, 0].astype(np.float64)
        minv = np.linalg.inv(cell[b].astype(np.float64))
        phi = rs @ minv                     # turns (unwrapped)
        dat = np.zeros((ATOMS_PER_CORE, CPC))
        p1 = np.outer(phi[:, 0], j1)
        dat[:, 0:22:2] = _cf(p1)            # h=0: sin-src
        dat[:, 1:22:2] = _cf(p1 - 0.25)     # h=1: cos-src
        dat[:, 22:43] = _cf(np.outer(phi[:, 1], n2r))
        gam = TWOPI * np.outer(phi[:, 2], n3r)
        dat[:, 43:64] = -qs[:, None] * np.sin(gam)
        dat[:, 64:85] = qs[:, None] * np.cos(gam)
        dat_p = np.zeros((PADN, CPC), np.float16)
        dat_p[:ATOMS_PER_CORE] = dat.astype(np.float16)
        # atom (t*128+p) -> row p, cols [t*CPC : (t+1)*CPC]
        inp = (
            dat_p.reshape(CHUNKS, 128, CPC).transpose(1, 0, 2).reshape(128, NIN)
        )
        in_maps.append({"inp": inp})
    return in_maps


def _host_weights(cell):
    """w[b, pair, n3] mirroring the reference's fp32 mask/kfac semantics."""
    k_sq_max = np.float32((TWOPI / DL) ** 2)
    ssh = np.float32(SIGMA ** 2 / 2.0)
    pairs = []
    for a0, a1, b0, b1 in BLOCKS:
        for n1 in range(a0, a1 + 1):
            for n2 in range(b0, b1 + 1):
                pairs.append((n1, n2))
    pairs = np.array(pairs)
    nvec = np.zeros((NP_PAIRS, 21, 3), np.float32)
    nvec[:, :, 0] = pairs[:, 0:1]
    nvec[:, :, 1] = pairs[:, 1:2]
    nvec[:, :, 2] = np.arange(-10, 11)[None, :]
    nflat = nvec.reshape(-1, 3)
    hemi = (
        (nflat[:, 0] > 0)
        | ((nflat[:, 0] == 0) & (nflat[:, 1] > 0))
        | ((nflat[:, 0] == 0) & (nflat[:, 1] == 0) & (nflat[:, 2] > 0))
    )
    ws = []
    for b in range(B):
        cb = cell[b]
        G = (np.float32(TWOPI) * np.linalg.inv(cb.astype(np.float64)).T).astype(
            np.float32
        )
        kvec = (nflat @ G).astype(np.float32)
        k_sq = np.sum(kvec * kvec, axis=1, dtype=np.float32)
        mask = (k_sq > 0) & (k_sq <= k_sq_max) & hemi
        kfac = np.exp(-ssh * k_sq) / (k_sq + np.float32(EPS))
        vol = np.float32(np.linalg.det(cb.astype(np.float64)))
        ws.append(np.where(mask, 2.0 * kfac, 0.0).astype(np.float64) / vol)
    return np.stack(ws).reshape(B, NP_PAIRS, 21)


def _col_maps():
    sin_col = np.zeros(NP_PAIRS, np.int64)
    cos_col = np.zeros(NP_PAIRS, np.int64)
    off = p = 0
    for a0, a1, b0, b1 in BLOCKS:
        na, nb = a1 - a0 + 1, b1 - b0 + 1
        for a in range(na):
            for bb in range(nb):
                sin_col[p] = off + (2 * a) * nb + bb
                cos_col[p] = off + (2 * a + 1) * nb + bb
                p += 1
        off += 2 * na * nb
    return sin_col, cos_col


def kernel(q, r, cell, batch):
    from concourse.bass_utils import run_bass_kernel_spmd

    q = np.asarray(q)
    r = np.asarray(r)
    cell = np.asarray(cell)

    nc = _get_nc()
    in_maps = _host_inputs(q, r, cell)
    res = run_bass_kernel_spmd(nc, in_maps, core_ids=list(range(N_CORES))).results

    w = _host_weights(cell)
    sin_col, cos_col = _col_maps()
    pot = np.zeros(B, np.float64)
    for b in range(B):
        P = (
            res[2 * b]["sout"].astype(np.float64)
            + res[2 * b + 1]["sout"].astype(np.float64)
        )
        S_r = P[21:42, :][:, cos_col].T - P[0:21, :][:, sin_col].T
        S_i = -P[21:42, :][:, sin_col].T - P[0:21, :][:, cos_col].T
        s_sq = S_r ** 2 + S_i ** 2
        qb = q[b * N_PER : (b + 1) * N_PER, 0].astype(np.float64)
        self_e = np.sum(qb ** 2) / (SIGMA * TWOPI ** 1.5)
        pot[b] = (np.sum(w[b] * s_sq) - self_e) * NORM
    return pot.astype(np.float32)
